# revision 40
# baseline (speedup 1.0000x reference)
"""Causal self-attention (B=4, S=2048, D=768, H=12) on 8 TRN2 NeuronCores.

Sharding: core = (batch b in 0..3) x (head-group hg in 0..1, 6 heads each).
Host pre-transposes x -> xT per batch, slices w_qkv columns / w_proj rows per
head-group.  Each core computes its 6 heads end-to-end and a partial
projection output [S, D]; the host sums the two head-group partials per batch
and adds b_proj plus the (attention-invariant) v-bias term b_v @ w_proj.

fp8 strategy: x, w_qkv, w_proj ship as fp8e4m3 (weights pre-scaled by 16 to
center them in the e4m3 normal range).  qkv-gen, v-gen, attn@V and proj
matmuls use fp8 DoubleRow perf mode.  Scores stay bf16 (contraction is 64).
Exp output is written fp8 by the ScalarE; causal masking is done AFTER exp by
zero-filling the invalid triangle bytes with gpsimd affine_select.  The 16*16
weight scale is divided out in the final projection copy.

Precision split: strip 0 (q<512) runs a bf16 path end-to-end (v for s-tiles
0-3 from bf16 x/w_v; strip-0 exp/attn@V/yT/proj bf16); later strips use fp8.

Scheduling: the ScalarE exp stream (~123us) is the pacing engine.  All
qkv/v/proj/copy work is emitted through an EDF work queue with per-(strip,
head-pair, chunk) deadlines so the PE filler spreads evenly instead of
bunching at strip boundaries.  Normalizes are deferred into the next strip.
w ships qk-pair-major so the first head-pair's weights arrive in one small
DMA; late-needed tensors (wp, xT[2:]) are DMA'd mid-kernel in program order
so Tile's DMA guards don't stall the first bias-adds.
"""

import numpy as np
from bisect import insort
from collections import deque
from contextlib import ExitStack

import concourse.bacc as bacc
import concourse.mybir as mybir
from concourse.tile import TileContext

F32 = mybir.dt.float32
BF16 = mybir.dt.bfloat16
FP8 = mybir.dt.float8e4
I8 = mybir.dt.int8
I16 = mybir.dt.int16

D = 768
NCORES = 8
SCALE = 0.125 / 256.0  # 1/sqrt(64) / (16*16 weight prescale)
INV_OUT = 1.0 / 256.0


def build_program(S=2048):
    NS = S // 512   # q strips
    NT = S // 128   # s tiles
    NC = NT // 2    # kb chunks (2 s-tiles each)
    DT = D // 128   # d tiles (contraction)
    DR = mybir.MatmulPerfMode.DoubleRow

    nc = bacc.Bacc()

    x8 = nc.dram_tensor("x8", [128, NS * DT * 512], FP8, kind="ExternalInput")
    w8 = nc.dram_tensor("w8", [128, DT, 1152], FP8, kind="ExternalInput")
    x0b = nc.dram_tensor("x0b", [128, DT * 256], BF16, kind="ExternalInput")
    wvb = nc.dram_tensor("wvb", [128, DT * 384], BF16, kind="ExternalInput")
    bqk = nc.dram_tensor("bqk_s", [128, 6], F32, kind="ExternalInput")
    wp8 = nc.dram_tensor("wp8", [128, 3 * D], FP8, kind="ExternalInput")
    wpb = nc.dram_tensor("wpb", [128, 3 * D], BF16, kind="ExternalInput")
    out = nc.dram_tensor("out_s", [S, D], BF16, kind="ExternalOutput")

    with TileContext(nc) as tc, ExitStack() as ctx:
        persist = ctx.enter_context(tc.tile_pool(name="persist", bufs=1))

        qkT = [persist.tile([128, S], BF16, tag=f"qkT{i}", name=f"qkT{i}")
               for i in range(6)]
        v_sb = [persist.tile([128, 2, 6, 80], FP8, tag=f"v{i}", name=f"v{i}")
                for i in range(NC)]
        vb_sb = [persist.tile([128, 2, 6, 65], BF16, tag=f"vb{i}",
                              name=f"vb{i}") for i in range(1)]
        yT = persist.tile([128, 3, S], FP8, tag="yT", name="yT")
        yTb = persist.tile([128, 3, 512], BF16, tag="yTb", name="yTb")
        wp = persist.tile([128, 3, D], FP8, tag="wp", name="wp")
        wpb_sb = persist.tile([128, 3, D], BF16, tag="wpb", name="wpb_sb")
        bqk_sb = persist.tile([128, 6], F32, tag="bqk", name="bqk_sb")
        dum_in = persist.tile([1, 8], F32, tag="dumi", name="dum_in")
        dum_out = persist.tile([128, 8], F32, tag="dumo", name="dum_out")

        xw_pool = ctx.enter_context(tc.tile_pool(name="xw", bufs=1))
        ps = ctx.enter_context(tc.tile_pool(name="ps", bufs=1, space="PSUM"))
        expp = ctx.enter_context(tc.tile_pool(name="expp", bufs=10))
        expb = ctx.enter_context(tc.tile_pool(name="expb", bufs=8))
        rcp = ctx.enter_context(tc.tile_pool(name="rcp", bufs=2))
        ytp = ctx.enter_context(tc.tile_pool(name="ytp", bufs=5))
        outp = ctx.enter_context(tc.tile_pool(name="outp", bufs=2))
        otac = ctx.enter_context(tc.tile_pool(name="otac", bufs=4))

        xT_sb = [xw_pool.tile([128, DT, 512], FP8, tag=f"xT{i}",
                              name=f"xTs{i}") for i in range(NS)]
        w_sb = xw_pool.tile([128, DT, 1152], FP8, tag="w", name="ws")
        x0b_sb = xw_pool.tile([128, DT, 256], BF16, tag="x0b", name="x0bs")
        wvb_sb = xw_pool.tile([128, DT, 384], BF16, tag="wvb", name="wvbs")

        # ---- early DMAs: exactly what the first score chunk needs, in
        # need order (each issue costs ~0.6us on the serial Sync queue).
        # w is packed qk-pair-major: cols [q0|k0|q1|k1|q2|k2|v(384)].
        nc.sync.dma_start(out=w_sb[:, :, 0:256], in_=w8[:, :, 0:256])
        nc.sync.dma_start(out=xT_sb[0][:, 0:2, :], in_=x8[:, 0:1024])
        nc.sync.dma_start(out=bqk_sb[:], in_=bqk[:])
        nc.sync.dma_start(out=xT_sb[0][:, 2:4, :], in_=x8[:, 1024:2048])
        nc.sync.dma_start(out=xT_sb[0][:, 4:6, :], in_=x8[:, 2048:3072])
        nc.sync.dma_start(out=x0b_sb[:], in_=x0b[:])
        nc.sync.dma_start(out=wvb_sb[:], in_=wvb[:])
        nc.sync.dma_start(out=xT_sb[1][:], in_=x8[:, 3072:6144])
        for c in range(NC):
            nc.vector.memset(v_sb[c][:, :, :, 64:65], 1.0)
        nc.vector.memset(vb_sb[0][:, :, :, 64:65], 1.0)
        # force the gpsimd partition_broadcast library load NOW, during the
        # input-DMA dead time (a mid-kernel UNLOAD/LOAD costs ~7us and
        # stalls the affine_select queue behind it)
        nc.gpsimd.memset(dum_in[:], 1.0)
        nc.gpsimd.partition_broadcast(dum_out[:], dum_in[:])

        # ---- phase work units ----
        def qk_cols(ct):
            c0 = 256 * (ct % 3) + 128 * (ct // 3)
            return c0, c0 + 128

        def p1_unit(ns, ct):
            # qkT[128ct..][strip ns] = (wqkv[:, qk cols].T @ xT) + bias
            c0, c1 = qk_cols(ct)
            psu = ps.tile([128, 512], F32, tag="mm", bufs=2, name="ps_qk")
            for i in range(DT // 2):
                nc.tensor.matmul(
                    psu[:],
                    w_sb[:, 2 * i:2 * i + 2, c0:c1],
                    xT_sb[ns][:, 2 * i:2 * i + 2, :],
                    start=(i == 0), stop=(i == DT // 2 - 1), perf_mode=DR)
            nc.vector.tensor_scalar_add(
                qkT[ct][:, 512 * ns:512 * ns + 512], psu[:],
                bqk_sb[:, ct:ct + 1])

        def p2_unit(st):
            # v for s-tile st (no bias: host folds b_v @ w_proj).
            psu = ps.tile([128, 384], F32, tag="mm", bufs=2, name="ps_v")
            if st < 2:
                for i in range(DT):
                    nc.tensor.matmul(
                        psu[:],
                        x0b_sb[:, i, 128 * st:128 * st + 128],
                        wvb_sb[:, i, :],
                        start=(i == 0), stop=(i == DT - 1))
                nc.vector.tensor_copy(
                    vb_sb[st // 2][:, st % 2, :, 0:64],
                    psu[:].rearrange("p (h e) -> p h e", h=6))
                return
            for i in range(DT // 2):
                nc.tensor.matmul(
                    psu[:],
                    xT_sb[st // 4][:, 2 * i:2 * i + 2,
                                   128 * (st % 4):128 * (st % 4) + 128],
                    w_sb[:, 2 * i:2 * i + 2, 768:1152],
                    start=(i == 0), stop=(i == DT // 2 - 1), perf_mode=DR)
            nc.vector.tensor_copy(
                v_sb[st // 2][:, st % 2, :, 0:64],
                psu[:].rearrange("p (h e) -> p h e", h=6))

        def p4_unit(st):
            # partial proj for s-tile st; divides out the 16*16 weight scale
            pa = ps.tile([128, 512], F32, tag="mm", bufs=2, name="pa")
            pb = ps.tile([128, 256], F32, tag="mm", bufs=2, name="pb")
            for p_, c0, c1 in ((pa, 0, 512), (pb, 512, 768)):
                if st < 2:
                    for yt in range(3):
                        nc.tensor.matmul(
                            p_[:], yTb[:, yt, 128 * st:128 * st + 128],
                            wpb_sb[:, yt, c0:c1],
                            start=(yt == 0), stop=(yt == 2))
                else:
                    nc.tensor.matmul(
                        p_[:], yT[:, 0:2, 128 * st:128 * st + 128],
                        wp[:, 0:2, c0:c1], start=True, stop=False,
                        perf_mode=DR)
                    nc.tensor.matmul(
                        p_[:], yT[:, 2, 128 * st:128 * st + 128],
                        wp[:, 2, c0:c1], start=False, stop=True)
            ot = outp.tile([128, D], BF16, tag="ot", name="ot")
            nc.vector.tensor_scalar_mul(ot[:, 0:512], pa[:], INV_OUT)
            nc.vector.tensor_scalar_mul(ot[:, 512:768], pb[:], INV_OUT)
            nc.sync.dma_start(out=out[128 * st:128 * st + 128, :], in_=ot[:])

        def v8_copy(st):
            # fp8 copy of the bf16-accumulated v for s-tiles 0-3
            nc.vector.tensor_copy(
                v_sb[st // 2][:, st % 2, :, 0:64],
                vb_sb[st // 2][:, st % 2, :, 0:64])

        ot_accs = {}

        def p4a_unit(st):
            # last-strip proj, hp0+hp1 partial: pre-scaled into an SBUF
            # accumulator so only the hp2 matmul remains after the final
            # normalize (shortens the post-last-exp tail)
            pa = ps.tile([128, 512], F32, tag="mm", bufs=2, name="pa")
            pb = ps.tile([128, 256], F32, tag="mm", bufs=2, name="pb")
            for p_, c0, c1 in ((pa, 0, 512), (pb, 512, 768)):
                nc.tensor.matmul(
                    p_[:], yT[:, 0:2, 128 * st:128 * st + 128],
                    wp[:, 0:2, c0:c1], start=True, stop=True,
                    perf_mode=DR)
            acc = otac.tile([128, D], BF16, tag="oa", name="oa", bufs=4)
            ot_accs[st % 4] = acc
            nc.vector.tensor_scalar_mul(acc[:, 0:512], pa[:], INV_OUT)
            nc.vector.tensor_scalar_mul(acc[:, 512:768], pb[:], INV_OUT)

        def p4b_unit(st):
            pa = ps.tile([128, 512], F32, tag="mm", bufs=2, name="pa")
            pb = ps.tile([128, 256], F32, tag="mm", bufs=2, name="pb")
            for p_, c0, c1 in ((pa, 0, 512), (pb, 512, 768)):
                nc.tensor.matmul(
                    p_[:], yT[:, 2, 128 * st:128 * st + 128],
                    wp[:, 2, c0:c1], start=True, stop=True)
            acc = ot_accs[st % 4]
            ot = outp.tile([128, D], BF16, tag="ot", name="ot")
            nc.vector.scalar_tensor_tensor(
                ot[:, 0:512], pa[:], INV_OUT, acc[:, 0:512],
                mybir.AluOpType.mult, mybir.AluOpType.add)
            nc.vector.scalar_tensor_tensor(
                ot[:, 512:768], pb[:], INV_OUT, acc[:, 512:768],
                mybir.AluOpType.mult, mybir.AluOpType.add)
            nc.sync.dma_start(out=out[128 * st:128 * st + 128, :], in_=ot[:])

        # ---- EDF work queue over a global integer-tick timeline.
        # Ordinals interleave strips 0/1 so strip-0's oversubscribed PE
        # work (bf16 v-gen, avs, tails) drains under strip-1's larger ACT
        # budget.  Each ordinal's flush (remaining attn@V + tail) is
        # deferred into the NEXT ordinal after its chunk-1 exps.
        ORDER = [(0, 0), (1, 0), (0, 1), (1, 1), (0, 2), (1, 2),
                 (2, 0), (2, 1), (2, 2), (3, 0), (3, 1), (3, 2)]
        NORD = len(ORDER)
        NCH = [2 * (ns + 1) for ns, hp in ORDER]
        base = [0] * NORD
        for o in range(1, NORD):
            base[o] = base[o - 1] + NCH[o - 1]

        def tick(o, c):
            return 10 * (base[o] + c)

        def F(o):  # flush key of ordinal o (runs inside ordinal o+1 at c1)
            if o + 1 < NORD:
                return tick(o + 1, 1) + 5
            return 10 * (base[NORD - 1] + NCH[NORD - 1]) + 5

        work = []
        wseq = [0]

        def add(key, fn, ready=-1):
            wseq[0] += 1
            insort(work, (key, wseq[0], ready, fn))

        def pop_due(now):
            while work and work[0][0] <= now:
                work.pop(0)[3]()

        def pop_one(now):
            for idx in range(len(work)):
                if work[idx][2] <= now:
                    work.pop(idx)[3]()
                    return

        ORD = {sh: i for i, sh in enumerate(ORDER)}
        rdy_x = {0: -1, 1: -1, 2: tick(4, 0), 3: tick(7, 0)}

        # p1: q-tile due at the previous ordinal's last-chunk pre;
        # own-strip k-tile due one chunk before the diagonal (c = 2ns).
        for ns in range(NS):
            for hp in range(3):
                o = ORD[(ns, hp)]
                if o == 0:
                    continue  # prologue
                kq = tick(o - 1, NCH[o - 1] - 1)
                add(kq, lambda a=ns, b=hp: p1_unit(a, b), ready=rdy_x[ns])
                kk = kq if ns == 0 else tick(o, 2 * ns - 1)
                add(kk, lambda a=ns, b=hp: p1_unit(a, 3 + b),
                    ready=rdy_x[ns])
        # p2: v_sb[cv] due one chunk before its first (deferred) av read.
        # Strip-0's bf16 units ride in ordinal 1's DMA-slack window.
        for st in range(NT):
            m = st // 4
            cv = st // 2
            if m == 0:
                key = tick(1, 0) if st < 2 else tick(1, 1)
                rdy = -1
            else:
                o = ORD[(m, 0)]
                key = tick(o, cv + 1) if cv + 1 < NCH[o] else F(o) - 1
                rdy = rdy_x[m]
            add(key, lambda a=st: p2_unit(a), ready=rdy)
        for st in range(2):
            key = tick(1, 1) + 6
            add(key, lambda a=st: v8_copy(a), ready=key)
        # p4: ready once all three of the strip's deferred norms are
        # emitted; deadlines spread over the later (ACT-slack) ordinals.
        p4sched = {0: (tick(6, 1), 7), 1: (tick(7, 1), 8),
                   2: (tick(10, 1), 10)}
        for ns in range(NS - 1):
            rdy, ko = p4sched[ns]
            for i in range(4):
                add(tick(ko, 2 + i), lambda a=4 * ns + i: p4_unit(a),
                    ready=rdy)
        for i in range(4):
            add(tick(NORD - 1, 3 + i), lambda a=12 + i: p4a_unit(a),
                ready=tick(NORD - 1, 2) + 5)

        # prologue: ordinal 0's q and k tiles, interleaved per dtile-pair
        # so the contraction chains consume the split x DMAs as they land
        psq = ps.tile([128, 512], F32, tag="mm", bufs=2, name="ps_qk")
        psk = ps.tile([128, 512], F32, tag="mm", bufs=2, name="ps_qk2")
        for i in range(DT // 2):
            for psu, ct in ((psq, 0), (psk, 3)):
                c0, c1 = qk_cols(ct)
                nc.tensor.matmul(
                    psu[:],
                    w_sb[:, 2 * i:2 * i + 2, c0:c1],
                    xT_sb[0][:, 2 * i:2 * i + 2, :],
                    start=(i == 0), stop=(i == DT // 2 - 1), perf_mode=DR)
        nc.vector.tensor_scalar_add(qkT[0][:, 0:512], psq[:],
                                    bqk_sb[:, 0:1])
        nc.vector.tensor_scalar_add(qkT[3][:, 0:512], psk[:],
                                    bqk_sb[:, 3:4])

        # ---- attention (ordinal-interleaved) ----
        flushes = {}
        for o, (ns, hp) in enumerate(ORDER):
            q0 = 512 * ns
            fp8_strip = ns > 0
            EXDT = FP8 if fp8_strip else BF16
            # late DMAs at fixed points, in program order so Tile's
            # conservative DMA guards land late too
            if o == 0:
                nc.sync.dma_start(out=w_sb[:, :, 256:512],
                                  in_=w8[:, :, 256:512])
                nc.sync.dma_start(out=w_sb[:, :, 512:768],
                                  in_=w8[:, :, 512:768])
                nc.sync.dma_start(out=w_sb[:, :, 768:1152],
                                  in_=w8[:, :, 768:1152])
            if o == 2:
                nc.sync.dma_start(out=wp[:], in_=wp8[:])
                nc.sync.dma_start(out=wpb_sb[:], in_=wpb[:])
            if o == 4:
                nc.sync.dma_start(out=xT_sb[2][:], in_=x8[:, 6144:9216])
            if o == 7:
                nc.sync.dma_start(out=xT_sb[3][:], in_=x8[:, 9216:12288])

            qt = qkT[hp]
            kt = qkT[3 + hp]
            nk = 4 * (ns + 1)
            nchunk = nk // 2
            yh = [ps.tile([65, 512], F32, tag="yh", bufs=2, name="yh0"),
                  ps.tile([65, 512], F32, tag="yh", bufs=2, name="yh1")]

            def emit_yT(c, ex_pair, c0, yh=yh, fp8_strip=fp8_strip,
                        hp=hp, nchunk=nchunk, nk=nk, q0=q0):
                for h in range(2):
                    if fp8_strip or c == 1:
                        nc.tensor.matmul(
                            yh[h][:, c0:512],
                            v_sb[c][:, :, 2 * hp + h, 0:65],
                            ex_pair[h][:, :, c0:512],
                            start=(c == 0), stop=(c == nchunk - 1),
                            perf_mode=DR, skip_group_check=True)
                    else:
                        for u in range(2):
                            kb = 2 * c + u
                            cu = max(0, 128 * kb - q0)
                            nc.tensor.matmul(
                                yh[h][:, cu:512],
                                vb_sb[c][:, u, 2 * hp + h, :],
                                ex_pair[h][:, u, cu:512],
                                start=(kb == 0), stop=(kb == nk - 1),
                                skip_group_check=True)

            prevs = deque()
            for c in range(nchunk):
                diag_c = c >= 2 * ns
                c0 = max(0, 256 * c - q0)
                fp8_c = fp8_strip or c == 1
                EXDT_c = FP8 if fp8_c else BF16
                ex_pair = []
                # two heads' score matmuls: distinct 64-row PE tiles and
                # PSUM banks, u-outer/h-inner so each (h0,h1) pair runs
                # concurrently in the split array
                scs = [ps.tile([128, 2, 512], F32, tag="sc", bufs=2,
                               name=f"sc2_{h}") for h in range(2)]
                for u in range(2):
                    kb = 2 * c + u
                    cu = max(0, 128 * kb - q0)
                    for h in range(2):
                        p0 = 64 * h
                        nc.tensor.matmul(
                            scs[h][:, u, cu:512],
                            kt[p0:p0 + 64, 128 * kb:128 * kb + 128],
                            qt[p0:p0 + 64, q0 + cu:q0 + 512],
                            start=True, stop=True)
                for h in range(2):
                    sc2 = scs[h]
                    pool = expp if fp8_c else expb
                    ex2 = pool.tile([128, 2, 512], EXDT_c, tag="exp",
                                    name="ex2")
                    nc.scalar.activation(
                        ex2[:, :, c0:512], sc2[:, :, c0:512],
                        mybir.ActivationFunctionType.Exp, scale=SCALE)
                    if diag_c:
                        # zero the causally-invalid bytes of the exp output
                        for u in range(2):
                            d = 2 * c + u - 4 * ns
                            z0, z1 = c0, min(512, 128 * d + 128)
                            if z1 <= z0:
                                continue
                            if z1 - z0 > 128:
                                # columns < 128d are invalid for every
                                # partition: plain memset (Vector), keep
                                # the gpsimd affine for the triangle only
                                nc.vector.memset(ex2[:, u, z0:z1 - 128], 0)
                                z0 = z1 - 128
                            idt = I8 if fp8_c else I16
                            ex_i = ex2[:, u, z0:z1].bitcast(idt)
                            nc.gpsimd.affine_select(
                                out=ex_i, in_=ex_i,
                                compare_op=mybir.AluOpType.is_ge,
                                fill=0, base=z0 - 128 * d,
                                pattern=[[1, z1 - z0]],
                                channel_multiplier=-1)
                    ex_pair.append(ex2)
                pop_due(tick(o, c))
                if o > 0 and c == min(2, nchunk - 1):
                    flushes.pop(o - 1)()  # prev ordinal's deferred flush
                pop_one(tick(o, c))
                if len(prevs) >= 2:
                    emit_yT(*prevs.popleft())
                prevs.append((c, ex_pair, c0))

            def make_flush(o=o, ns=ns, hp=hp, q0=q0, prevs=prevs,
                           emit_yT=emit_yT, yh=yh):
                def flush():
                    pop_due(F(o))
                    while prevs:
                        emit_yT(*prevs.popleft())
                    # tail: stage yh to SBUF fast, recip the denominator
                    # row, gpsimd-broadcast it (library preloaded in the
                    # prologue); norms deferred ~one ordinal further out
                    yst = (yTb[:, hp, :] if ns == 0
                           else yT[:, hp, q0:q0 + 512])
                    ytmp = ytp.tile([128, 512], BF16, tag="ytmp",
                                    name="ytmp")
                    for h in range(2):
                        lrow = rcp.tile([1, 512], F32, tag="lrow",
                                        name="lrow", bufs=8)
                        nc.vector.tensor_copy(ytmp[64 * h:64 * h + 64, :],
                                              yh[h][0:64, :])
                        nc.vector.tensor_copy(lrow[:], yh[h][64:65, :])
                        rec = rcp.tile([1, 512], F32, tag="rec",
                                       name="rec", bufs=8)
                        nc.vector.reciprocal_approx_fast(rec[:], lrow[:])
                        rb = rcp.tile([128, 512], F32, tag="rb", bufs=8,
                                      name="rb")
                        nc.gpsimd.partition_broadcast(rb[:], rec[:])

                        def norm(h=h, ytmp=ytmp, yst=yst, rb=rb, ns=ns,
                                 hp=hp):
                            if ns == 0:
                                nc.vector.tensor_mul(
                                    yst[64 * h:64 * h + 64, 0:256],
                                    ytmp[64 * h:64 * h + 64, 0:256],
                                    rb[64 * h:64 * h + 64, 0:256])
                                nc.vector.tensor_mul(
                                    yT[64 * h:64 * h + 64, hp, 256:512],
                                    ytmp[64 * h:64 * h + 64, 256:512],
                                    rb[64 * h:64 * h + 64, 256:512])
                            else:
                                nc.vector.tensor_mul(
                                    yst[64 * h:64 * h + 64, :],
                                    ytmp[64 * h:64 * h + 64, :],
                                    rb[64 * h:64 * h + 64, :])
                        nkey = (tick(o + 2, 0) + h if o + 2 < NORD
                                else tick(NORD - 1, 2) + h)
                        add(nkey, norm)
                return flush

            if o < NORD - 1:
                flushes[o] = make_flush()
            else:
                # last ordinal: flush inline, interleaving the final
                # normalizes with the last strip's proj units
                pop_due(F(o))
                while prevs:
                    emit_yT(*prevs.popleft())
                yst = yT[:, hp, q0:q0 + 512]
                ytmp = ytp.tile([128, 512], BF16, tag="ytmp", name="ytmp")
                rbs = []
                for h in range(2):
                    lrow = rcp.tile([1, 512], F32, tag="lrow", name="lrow",
                                    bufs=8)
                    nc.scalar.copy(ytmp[64 * h:64 * h + 64, :],
                                   yh[h][0:64, :])
                    nc.scalar.copy(lrow[:], yh[h][64:65, :])
                    rec = rcp.tile([1, 512], F32, tag="rec", name="rec",
                                   bufs=8)
                    nc.vector.reciprocal_approx_fast(rec[:], lrow[:])
                    rb = rcp.tile([128, 512], F32, tag="rb", bufs=8,
                                  name="rb")
                    rbs.append(rb)
                    nc.gpsimd.partition_broadcast(rb[:], rec[:])
                for qc in range(4):
                    cl, cr = 128 * qc, 128 * qc + 128
                    for h in range(2):
                        nc.vector.tensor_mul(
                            yst[64 * h:64 * h + 64, cl:cr],
                            ytmp[64 * h:64 * h + 64, cl:cr],
                            rbs[h][64 * h:64 * h + 64, cl:cr])
                    p4b_unit(4 * ns + qc)
        while work:
            work.pop(0)[3]()

    nc.finalize()
    return nc


def shard_inputs(x, w_qkv, b_qkv, w_proj):
    """Host-side sharding: returns list of per-core input dicts.

      x8  [128, ns, d, s]  fp8   w8  [128, d, 1152] fp8 (x16, qk-pair-major)
      x0b [128, d, s0]     bf16  wvb [128, d, 384]  bf16 (x16)
      wp8/wpb [128, 3, 768] (x16), bqk [128, 6] f32 (x16)
    """
    import ml_dtypes
    E4M3 = ml_dtypes.float8_e4m3fn
    BF = ml_dtypes.bfloat16
    S16 = np.float32(16.0)
    in_maps = []
    for core in range(NCORES):
        b, hg = (core // 2) % x.shape[0], core % 2
        cs = slice(384 * hg, 384 * hg + 384)
        xT_s = np.ascontiguousarray(x[b].T).astype(np.float32)  # [768, 2048]
        q_s = w_qkv[:, 0:768][:, cs]
        k_s = w_qkv[:, 768:1536][:, cs]
        v_s = w_qkv[:, 1536:2304][:, cs]
        # qk-pair-major: [q0|k0|q1|k1|q2|k2|v]
        wqkv_s = np.concatenate(
            [q_s[:, 0:128], k_s[:, 0:128], q_s[:, 128:256], k_s[:, 128:256],
             q_s[:, 256:384], k_s[:, 256:384], v_s], axis=1) * S16
        bqk_c = np.concatenate([b_qkv[0:768][cs], b_qkv[768:1536][cs]])
        bqk_s = np.ascontiguousarray(bqk_c.reshape(6, 128).T) * S16
        wproj_s = w_proj[384 * hg:384 * hg + 384, :] * S16  # [384, 768]

        x8 = xT_s.reshape(6, 128, 4, 512).transpose(1, 2, 0, 3)
        w8 = wqkv_s.reshape(6, 128, 1152).transpose(1, 0, 2)
        x0b = xT_s[:, 0:256].reshape(6, 128, 256).transpose(1, 0, 2)
        wvb_h = wqkv_s[:, 768:1152].reshape(6, 128, 384).transpose(1, 0, 2)
        wpp = wproj_s.reshape(3, 128, 768).transpose(1, 0, 2)
        in_maps.append({
            "x8": np.ascontiguousarray(x8).astype(E4M3).reshape(128, -1),
            "w8": np.ascontiguousarray(w8).astype(E4M3),
            "x0b": np.ascontiguousarray(x0b).astype(BF).reshape(128, -1),
            "wvb": np.ascontiguousarray(wvb_h).astype(BF).reshape(128, -1),
            "bqk_s": bqk_s.astype(np.float32),
            "wp8": np.ascontiguousarray(wpp).astype(E4M3).reshape(128, -1),
            "wpb": np.ascontiguousarray(wpp).astype(BF).reshape(128, -1),
        })
    return in_maps


_CACHED = {}


def _get_program():
    if "nc" not in _CACHED:
        _CACHED["nc"] = build_program()
    return _CACHED["nc"]


def _spot_check(outp, x, w_qkv, b_qkv, w_proj, b_proj):
    """Exact per-row reference on a few rows; returns worst relative error.
    Guards against rare transient bad compiles/executions."""
    B, S, dim = x.shape
    H, HD = 12, 64
    worst = 0.0
    checks = [(b, min(S - 1, 511 + 512 * b)) for b in range(B)]
    checks += [(0, 5), (1, 300), (2, 1200), (3, 1800)]
    for b, s in checks:
        xb = x[b].astype(np.float64)
        q = xb[s] @ w_qkv[:, 0:768] + b_qkv[0:768]
        k = xb[:s + 1] @ w_qkv[:, 768:1536] + b_qkv[768:1536]
        v = xb[:s + 1] @ w_qkv[:, 1536:2304] + b_qkv[1536:2304]
        ys = []
        for h in range(H):
            sc = (k[:, HD * h:HD * h + HD] @ q[HD * h:HD * h + HD]) * 0.125
            e = np.exp(sc - sc.max())
            ys.append((e / e.sum()) @ v[:, HD * h:HD * h + HD])
        row = np.concatenate(ys) @ w_proj + b_proj
        rel = np.abs(outp[b, s] - row).max() / max(np.abs(row).max(), 1e-6)
        worst = max(worst, rel)
    return worst


def kernel(x, w_qkv, b_qkv, w_proj, b_proj):
    import jax
    from concourse.bass_utils import run_bass_kernel_spmd

    x = np.asarray(x, dtype=np.float32)
    w_qkv = np.asarray(w_qkv, dtype=np.float32)
    b_qkv = np.asarray(b_qkv, dtype=np.float32)
    w_proj = np.asarray(w_proj, dtype=np.float32)
    b_proj = np.asarray(b_proj, dtype=np.float32)

    B, S, dim = x.shape
    in_maps = shard_inputs(x, w_qkv, b_qkv, w_proj)
    # v-bias folds out of attention (rows of attn sum to exactly 1)
    bvw = b_qkv[1536:2304] @ w_proj  # [D]
    const_row = (b_proj + bvw)[None, :]

    outp = np.empty((B, S, dim), dtype=np.float32)
    for attempt in range(3):
        nc = _get_program()
        res = run_bass_kernel_spmd(nc, in_maps, core_ids=list(range(NCORES)))
        parts = [m["out_s"] for m in res.results]
        for b in range(B):
            outp[b] = parts[2 * b] + parts[2 * b + 1] + const_row
        if _spot_check(outp, x, w_qkv, b_qkv, w_proj, b_proj) < 1.2e-2:
            break
        # transient bad build/execution: clear caches, rebuild, rerun
        _CACHED.clear()
        jax.clear_caches()
    return outp


# revision 42
# speedup vs baseline: 1.0288x; 1.0288x over previous
"""Causal self-attention (B=4, S=2048, D=768, H=12) on 8 TRN2 NeuronCores.

Sharding: core = (batch b in 0..3) x (head-group hg in 0..1, 6 heads each).
Host pre-transposes x -> xT per batch, slices w_qkv columns / w_proj rows per
head-group.  Each core computes its 6 heads end-to-end and a partial
projection output [S, D]; the host sums the two head-group partials per batch
and adds b_proj plus the (attention-invariant) v-bias term b_v @ w_proj.

fp8 strategy: x, w_qkv, w_proj ship as fp8e4m3 (weights pre-scaled by 16 to
center them in the e4m3 normal range).  qkv-gen, v-gen, attn@V and proj
matmuls use fp8 DoubleRow perf mode.  Scores stay bf16 (contraction is 64).
Exp output is written fp8 by the ScalarE; causal masking is done AFTER exp by
zero-filling the invalid triangle bytes with gpsimd affine_select.  The 16*16
weight scale is divided out in the final projection copy.

Precision split: strip 0 (q<512) runs a bf16 path end-to-end (v for s-tiles
0-3 from bf16 x/w_v; strip-0 exp/attn@V/yT/proj bf16); later strips use fp8.

Scheduling: the ScalarE exp stream (~123us) is the pacing engine.  All
qkv/v/proj/copy work is emitted through an EDF work queue with per-(strip,
head-pair, chunk) deadlines so the PE filler spreads evenly instead of
bunching at strip boundaries.  Normalizes are deferred into the next strip.
w ships qk-pair-major so the first head-pair's weights arrive in one small
DMA; late-needed tensors (wp, xT[2:]) are DMA'd mid-kernel in program order
so Tile's DMA guards don't stall the first bias-adds.
"""

import numpy as np
from bisect import insort
from collections import deque
from contextlib import ExitStack

import concourse.bacc as bacc
import concourse.mybir as mybir
from concourse.tile import TileContext

F32 = mybir.dt.float32
BF16 = mybir.dt.bfloat16
FP8 = mybir.dt.float8e4
I8 = mybir.dt.int8
I16 = mybir.dt.int16

D = 768
NCORES = 8
SCALE = 0.125 / 256.0  # 1/sqrt(64) / (16*16 weight prescale)
INV_OUT = 1.0 / 256.0


def build_program(S=2048):
    NS = S // 512   # q strips
    NT = S // 128   # s tiles
    NC = NT // 2    # kb chunks (2 s-tiles each)
    DT = D // 128   # d tiles (contraction)
    DR = mybir.MatmulPerfMode.DoubleRow

    nc = bacc.Bacc()

    x8 = nc.dram_tensor("x8", [128, NS * DT * 512], FP8, kind="ExternalInput")
    w8 = nc.dram_tensor("w8", [128, DT, 1152], FP8, kind="ExternalInput")
    x0b = nc.dram_tensor("x0b", [128, DT * 256], BF16, kind="ExternalInput")
    wvb = nc.dram_tensor("wvb", [128, DT * 384], BF16, kind="ExternalInput")
    bqk = nc.dram_tensor("bqk_s", [128, 6], F32, kind="ExternalInput")
    wp8 = nc.dram_tensor("wp8", [128, 3 * D], FP8, kind="ExternalInput")
    wpb = nc.dram_tensor("wpb", [128, 3 * D], BF16, kind="ExternalInput")
    out = nc.dram_tensor("out_s", [S, D], BF16, kind="ExternalOutput")

    with TileContext(nc) as tc, ExitStack() as ctx:
        persist = ctx.enter_context(tc.tile_pool(name="persist", bufs=1))

        qkT = [persist.tile([128, S], BF16, tag=f"qkT{i}", name=f"qkT{i}")
               for i in range(6)]
        v_sb = [persist.tile([128, 2, 6, 80], FP8, tag=f"v{i}", name=f"v{i}")
                for i in range(NC)]
        vb_sb = [persist.tile([128, 2, 6, 65], BF16, tag=f"vb{i}",
                              name=f"vb{i}") for i in range(1)]
        yT = persist.tile([128, 3, S], FP8, tag="yT", name="yT")
        yTb = persist.tile([128, 3, 512], BF16, tag="yTb", name="yTb")
        wp = persist.tile([128, 3, D], FP8, tag="wp", name="wp")
        wpb_sb = persist.tile([128, 3, D], BF16, tag="wpb", name="wpb_sb")
        bqk_sb = persist.tile([128, 6], F32, tag="bqk", name="bqk_sb")
        dum_in = persist.tile([1, 8], F32, tag="dumi", name="dum_in")
        dum_out = persist.tile([128, 8], F32, tag="dumo", name="dum_out")

        xw_pool = ctx.enter_context(tc.tile_pool(name="xw", bufs=1))
        ps = ctx.enter_context(tc.tile_pool(name="ps", bufs=1, space="PSUM"))
        expp = ctx.enter_context(tc.tile_pool(name="expp", bufs=10))
        expb = ctx.enter_context(tc.tile_pool(name="expb", bufs=8))
        rcp = ctx.enter_context(tc.tile_pool(name="rcp", bufs=2))
        ytp = ctx.enter_context(tc.tile_pool(name="ytp", bufs=5))
        outp = ctx.enter_context(tc.tile_pool(name="outp", bufs=2))
        otac = ctx.enter_context(tc.tile_pool(name="otac", bufs=4))

        xT_sb = [xw_pool.tile([128, DT, 512], FP8, tag=f"xT{i}",
                              name=f"xTs{i}") for i in range(NS)]
        w_sb = xw_pool.tile([128, DT, 1152], FP8, tag="w", name="ws")
        x0b_sb = xw_pool.tile([128, DT, 256], BF16, tag="x0b", name="x0bs")
        wvb_sb = xw_pool.tile([128, DT, 384], BF16, tag="wvb", name="wvbs")

        # ---- early DMAs: exactly what the first score chunk needs, in
        # need order (each issue costs ~0.6us on the serial Sync queue).
        # w is packed qk-pair-major: cols [q0|k0|q1|k1|q2|k2|v(384)].
        nc.sync.dma_start(out=w_sb[:, :, 0:256], in_=w8[:, :, 0:256])
        nc.sync.dma_start(out=xT_sb[0][:, 0:2, :], in_=x8[:, 0:1024])
        nc.sync.dma_start(out=bqk_sb[:], in_=bqk[:])
        nc.sync.dma_start(out=xT_sb[0][:, 2:4, :], in_=x8[:, 1024:2048])
        nc.sync.dma_start(out=xT_sb[0][:, 4:6, :], in_=x8[:, 2048:3072])
        nc.sync.dma_start(out=x0b_sb[:], in_=x0b[:])
        nc.sync.dma_start(out=wvb_sb[:], in_=wvb[:])
        nc.sync.dma_start(out=xT_sb[1][:], in_=x8[:, 3072:6144])
        for c in range(NC):
            nc.vector.memset(v_sb[c][:, :, :, 64:65], 1.0)
        nc.vector.memset(vb_sb[0][:, :, :, 64:65], 1.0)
        # force the gpsimd partition_broadcast library load NOW, during the
        # input-DMA dead time (a mid-kernel UNLOAD/LOAD costs ~7us and
        # stalls the affine_select queue behind it)
        nc.gpsimd.memset(dum_in[:], 1.0)
        nc.gpsimd.partition_broadcast(dum_out[:], dum_in[:])

        # ---- phase work units ----
        def qk_cols(ct):
            c0 = 256 * (ct % 3) + 128 * (ct // 3)
            return c0, c0 + 128

        def p1_unit(ns, ct):
            # qkT[128ct..][strip ns] = (wqkv[:, qk cols].T @ xT) + bias
            c0, c1 = qk_cols(ct)
            psu = ps.tile([128, 512], F32, tag="mm", bufs=2, name="ps_qk")
            for i in range(DT // 2):
                nc.tensor.matmul(
                    psu[:],
                    w_sb[:, 2 * i:2 * i + 2, c0:c1],
                    xT_sb[ns][:, 2 * i:2 * i + 2, :],
                    start=(i == 0), stop=(i == DT // 2 - 1), perf_mode=DR)
            nc.vector.tensor_scalar_add(
                qkT[ct][:, 512 * ns:512 * ns + 512], psu[:],
                bqk_sb[:, ct:ct + 1])

        def p2_unit(st):
            # v for s-tile st (no bias: host folds b_v @ w_proj).
            psu = ps.tile([128, 384], F32, tag="mm", bufs=2, name="ps_v")
            if st < 2:
                for i in range(DT):
                    nc.tensor.matmul(
                        psu[:],
                        x0b_sb[:, i, 128 * st:128 * st + 128],
                        wvb_sb[:, i, :],
                        start=(i == 0), stop=(i == DT - 1))
                nc.vector.tensor_copy(
                    vb_sb[st // 2][:, st % 2, :, 0:64],
                    psu[:].rearrange("p (h e) -> p h e", h=6))
                return
            for i in range(DT // 2):
                nc.tensor.matmul(
                    psu[:],
                    xT_sb[st // 4][:, 2 * i:2 * i + 2,
                                   128 * (st % 4):128 * (st % 4) + 128],
                    w_sb[:, 2 * i:2 * i + 2, 768:1152],
                    start=(i == 0), stop=(i == DT // 2 - 1), perf_mode=DR)
            nc.vector.tensor_copy(
                v_sb[st // 2][:, st % 2, :, 0:64],
                psu[:].rearrange("p (h e) -> p h e", h=6))

        def p4_unit(st):
            # partial proj for s-tile st; divides out the 16*16 weight scale
            pa = ps.tile([128, 512], F32, tag="mm", bufs=2, name="pa")
            pb = ps.tile([128, 256], F32, tag="mm", bufs=2, name="pb")
            for p_, c0, c1 in ((pa, 0, 512), (pb, 512, 768)):
                if st < 2:
                    for yt in range(3):
                        nc.tensor.matmul(
                            p_[:], yTb[:, yt, 128 * st:128 * st + 128],
                            wpb_sb[:, yt, c0:c1],
                            start=(yt == 0), stop=(yt == 2))
                else:
                    nc.tensor.matmul(
                        p_[:], yT[:, 0:2, 128 * st:128 * st + 128],
                        wp[:, 0:2, c0:c1], start=True, stop=False,
                        perf_mode=DR)
                    nc.tensor.matmul(
                        p_[:], yT[:, 2, 128 * st:128 * st + 128],
                        wp[:, 2, c0:c1], start=False, stop=True)
            ot = outp.tile([128, D], BF16, tag="ot", name="ot")
            nc.vector.tensor_scalar_mul(ot[:, 0:512], pa[:], INV_OUT)
            nc.vector.tensor_scalar_mul(ot[:, 512:768], pb[:], INV_OUT)
            nc.sync.dma_start(out=out[128 * st:128 * st + 128, :], in_=ot[:])

        def v8_copy(st):
            # fp8 copy of the bf16-accumulated v for s-tiles 0-3
            nc.vector.tensor_copy(
                v_sb[st // 2][:, st % 2, :, 0:64],
                vb_sb[st // 2][:, st % 2, :, 0:64])

        ot_accs = {}

        def p4a_unit(st):
            # last-strip proj, hp0+hp1 partial: pre-scaled into an SBUF
            # accumulator so only the hp2 matmul remains after the final
            # normalize (shortens the post-last-exp tail)
            pa = ps.tile([128, 512], F32, tag="mm", bufs=2, name="pa")
            pb = ps.tile([128, 256], F32, tag="mm", bufs=2, name="pb")
            for p_, c0, c1 in ((pa, 0, 512), (pb, 512, 768)):
                nc.tensor.matmul(
                    p_[:], yT[:, 0:2, 128 * st:128 * st + 128],
                    wp[:, 0:2, c0:c1], start=True, stop=True,
                    perf_mode=DR)
            acc = otac.tile([128, D], BF16, tag="oa", name="oa", bufs=4)
            ot_accs[st % 4] = acc
            nc.vector.tensor_scalar_mul(acc[:, 0:512], pa[:], INV_OUT)
            nc.vector.tensor_scalar_mul(acc[:, 512:768], pb[:], INV_OUT)

        def p4b_unit(st):
            pa = ps.tile([128, 512], F32, tag="mm", bufs=2, name="pa")
            pb = ps.tile([128, 256], F32, tag="mm", bufs=2, name="pb")
            for p_, c0, c1 in ((pa, 0, 512), (pb, 512, 768)):
                nc.tensor.matmul(
                    p_[:], yT[:, 2, 128 * st:128 * st + 128],
                    wp[:, 2, c0:c1], start=True, stop=True)
            acc = ot_accs[st % 4]
            ot = outp.tile([128, D], BF16, tag="ot", name="ot")
            nc.vector.scalar_tensor_tensor(
                ot[:, 0:512], pa[:], INV_OUT, acc[:, 0:512],
                mybir.AluOpType.mult, mybir.AluOpType.add)
            nc.vector.scalar_tensor_tensor(
                ot[:, 512:768], pb[:], INV_OUT, acc[:, 512:768],
                mybir.AluOpType.mult, mybir.AluOpType.add)
            nc.sync.dma_start(out=out[128 * st:128 * st + 128, :], in_=ot[:])

        # ---- EDF work queue over a global integer-tick timeline.
        # Ordinals interleave strips 0/1 so strip-0's oversubscribed PE
        # work (bf16 v-gen, avs, tails) drains under strip-1's larger ACT
        # budget.  Each ordinal's flush (remaining attn@V + tail) is
        # deferred into the NEXT ordinal after its chunk-1 exps.
        ORDER = [(0, 0), (1, 0), (0, 1), (1, 1), (0, 2), (1, 2),
                 (2, 0), (2, 1), (2, 2), (3, 0), (3, 1), (3, 2)]
        NORD = len(ORDER)
        NCH = [2 * (ns + 1) for ns, hp in ORDER]
        base = [0] * NORD
        for o in range(1, NORD):
            base[o] = base[o - 1] + NCH[o - 1]

        def tick(o, c):
            return 10 * (base[o] + c)

        def F(o):  # flush key of ordinal o (runs inside ordinal o+1 at c1)
            if o + 1 < NORD:
                return tick(o + 1, 1) + 5
            return 10 * (base[NORD - 1] + NCH[NORD - 1]) + 5

        work = []
        wseq = [0]

        def add(key, fn, ready=-1):
            wseq[0] += 1
            insort(work, (key, wseq[0], ready, fn))

        def pop_due(now):
            while work and work[0][0] <= now:
                work.pop(0)[3]()

        def pop_one(now):
            for idx in range(len(work)):
                if work[idx][2] <= now:
                    work.pop(idx)[3]()
                    return

        ORD = {sh: i for i, sh in enumerate(ORDER)}
        rdy_x = {0: -1, 1: -1, 2: tick(4, 0), 3: tick(7, 0)}

        # p1: q-tile due at the previous ordinal's last-chunk pre;
        # own-strip k-tile due one chunk before the diagonal (c = 2ns).
        for ns in range(NS):
            for hp in range(3):
                o = ORD[(ns, hp)]
                if o == 0:
                    continue  # prologue
                kq = tick(o - 1, NCH[o - 1] - 1)
                add(kq, lambda a=ns, b=hp: p1_unit(a, b), ready=rdy_x[ns])
                kk = kq if ns == 0 else tick(o, 2 * ns - 1)
                add(kk, lambda a=ns, b=hp: p1_unit(a, 3 + b),
                    ready=rdy_x[ns])
        # p2: v_sb[cv] due one chunk before its first (deferred) av read.
        # Strip-0's bf16 units ride in ordinal 1's DMA-slack window.
        for st in range(NT):
            m = st // 4
            cv = st // 2
            if m == 0:
                key = tick(1, 0) if st < 2 else tick(1, 1)
                rdy = -1
            else:
                o = ORD[(m, 0)]
                key = tick(o, cv + 1) if cv + 1 < NCH[o] else F(o) - 1
                rdy = rdy_x[m]
            add(key, lambda a=st: p2_unit(a), ready=rdy)
        for st in range(2):
            key = tick(1, 1) + 6
            add(key, lambda a=st: v8_copy(a), ready=key)
        # p4: ready once all three of the strip's deferred norms are
        # emitted; deadlines spread over the later (ACT-slack) ordinals.
        p4sched = {0: (tick(6, 1), 7), 1: (tick(7, 1), 8),
                   2: (tick(10, 1), 10)}
        for ns in range(NS - 1):
            rdy, ko = p4sched[ns]
            for i in range(4):
                add(tick(ko, 2 + i), lambda a=4 * ns + i: p4_unit(a),
                    ready=rdy)
        for i in range(4):
            add(tick(NORD - 1, 4 + i), lambda a=12 + i: p4a_unit(a),
                ready=tick(NORD - 1, 2) + 5)

        # prologue: ordinal 0's q and k tiles, interleaved per dtile-pair
        # so the contraction chains consume the split x DMAs as they land
        psq = ps.tile([128, 512], F32, tag="mm", bufs=2, name="ps_qk")
        psk = ps.tile([128, 512], F32, tag="mm", bufs=2, name="ps_qk2")
        for i in range(DT // 2):
            for psu, ct in ((psq, 0), (psk, 3)):
                c0, c1 = qk_cols(ct)
                nc.tensor.matmul(
                    psu[:],
                    w_sb[:, 2 * i:2 * i + 2, c0:c1],
                    xT_sb[0][:, 2 * i:2 * i + 2, :],
                    start=(i == 0), stop=(i == DT // 2 - 1), perf_mode=DR)
        nc.vector.tensor_scalar_add(qkT[0][:, 0:512], psq[:],
                                    bqk_sb[:, 0:1])
        nc.vector.tensor_scalar_add(qkT[3][:, 0:512], psk[:],
                                    bqk_sb[:, 3:4])

        # ---- attention (ordinal-interleaved) ----
        flushes = {}
        for o, (ns, hp) in enumerate(ORDER):
            q0 = 512 * ns
            fp8_strip = ns > 0
            EXDT = FP8 if fp8_strip else BF16
            # late DMAs at fixed points, in program order so Tile's
            # conservative DMA guards land late too
            if o == 0:
                nc.sync.dma_start(out=w_sb[:, :, 256:512],
                                  in_=w8[:, :, 256:512])
                nc.sync.dma_start(out=w_sb[:, :, 512:768],
                                  in_=w8[:, :, 512:768])
                nc.sync.dma_start(out=w_sb[:, :, 768:1152],
                                  in_=w8[:, :, 768:1152])
            if o == 2:
                nc.sync.dma_start(out=wp[:], in_=wp8[:])
                nc.sync.dma_start(out=wpb_sb[:], in_=wpb[:])
            if o == 4:
                nc.sync.dma_start(out=xT_sb[2][:], in_=x8[:, 6144:9216])
            if o == 7:
                nc.sync.dma_start(out=xT_sb[3][:], in_=x8[:, 9216:12288])

            qt = qkT[hp]
            kt = qkT[3 + hp]
            nk = 4 * (ns + 1)
            nchunk = nk // 2
            yh = [ps.tile([65, 512], F32, tag="yh", bufs=2, name="yh0"),
                  ps.tile([65, 512], F32, tag="yh", bufs=2, name="yh1")]

            def emit_yT(c, ex_pair, c0, yh=yh, fp8_strip=fp8_strip,
                        hp=hp, nchunk=nchunk, nk=nk, q0=q0):
                for h in range(2):
                    if fp8_strip or c == 1:
                        nc.tensor.matmul(
                            yh[h][:, c0:512],
                            v_sb[c][:, :, 2 * hp + h, 0:65],
                            ex_pair[h][:, :, c0:512],
                            start=(c == 0), stop=(c == nchunk - 1),
                            perf_mode=DR, skip_group_check=True)
                    else:
                        for u in range(2):
                            kb = 2 * c + u
                            cu = max(0, 128 * kb - q0)
                            nc.tensor.matmul(
                                yh[h][:, cu:512],
                                vb_sb[c][:, u, 2 * hp + h, :],
                                ex_pair[h][:, u, cu:512],
                                start=(kb == 0), stop=(kb == nk - 1),
                                skip_group_check=True)

            prevs = deque()
            for c in range(nchunk):
                diag_c = c >= 2 * ns
                c0 = max(0, 256 * c - q0)
                fp8_c = fp8_strip or c == 1
                EXDT_c = FP8 if fp8_c else BF16
                pop_due(tick(o, c))
                ex_pair = []
                # two heads' score matmuls: distinct 64-row PE tiles and
                # PSUM banks, u-outer/h-inner so each (h0,h1) pair runs
                # concurrently in the split array
                scs = [ps.tile([128, 2, 512], F32, tag="sc", bufs=2,
                               name=f"sc2_{h}") for h in range(2)]
                for u in range(2):
                    kb = 2 * c + u
                    cu = max(0, 128 * kb - q0)
                    for h in range(2):
                        p0 = 64 * h
                        nc.tensor.matmul(
                            scs[h][:, u, cu:512],
                            kt[p0:p0 + 64, 128 * kb:128 * kb + 128],
                            qt[p0:p0 + 64, q0 + cu:q0 + 512],
                            start=True, stop=True)
                for h in range(2):
                    sc2 = scs[h]
                    pool = expp if fp8_c else expb
                    ex2 = pool.tile([128, 2, 512], EXDT_c, tag="exp",
                                    name="ex2")
                    nc.scalar.activation(
                        ex2[:, :, c0:512], sc2[:, :, c0:512],
                        mybir.ActivationFunctionType.Exp, scale=SCALE)
                    if diag_c:
                        # zero the causally-invalid bytes of the exp output
                        for u in range(2):
                            d = 2 * c + u - 4 * ns
                            z0, z1 = c0, min(512, 128 * d + 128)
                            if z1 <= z0:
                                continue
                            if z1 - z0 > 128:
                                # columns < 128d are invalid for every
                                # partition: plain memset (Vector), keep
                                # the gpsimd affine for the triangle only
                                nc.vector.memset(ex2[:, u, z0:z1 - 128], 0)
                                z0 = z1 - 128
                            idt = I8 if fp8_c else I16
                            ex_i = ex2[:, u, z0:z1].bitcast(idt)
                            nc.gpsimd.affine_select(
                                out=ex_i, in_=ex_i,
                                compare_op=mybir.AluOpType.is_ge,
                                fill=0, base=z0 - 128 * d,
                                pattern=[[1, z1 - z0]],
                                channel_multiplier=-1)
                    ex_pair.append(ex2)
                if o > 0 and c == min(2, nchunk - 1):
                    flushes.pop(o - 1)()  # prev ordinal's deferred flush
                pop_one(tick(o, c))
                if len(prevs) >= 2:
                    emit_yT(*prevs.popleft())
                prevs.append((c, ex_pair, c0))

            def make_flush(o=o, ns=ns, hp=hp, q0=q0, prevs=prevs,
                           emit_yT=emit_yT, yh=yh):
                def flush():
                    pop_due(F(o))
                    while prevs:
                        emit_yT(*prevs.popleft())
                    # tail: stage yh to SBUF fast, recip the denominator
                    # row, gpsimd-broadcast it (library preloaded in the
                    # prologue); norms deferred ~one ordinal further out
                    yst = (yTb[:, hp, :] if ns == 0
                           else yT[:, hp, q0:q0 + 512])
                    ytmp = ytp.tile([128, 512], BF16, tag="ytmp",
                                    name="ytmp")
                    for h in range(2):
                        lrow = rcp.tile([1, 512], F32, tag="lrow",
                                        name="lrow", bufs=8)
                        nc.vector.tensor_copy(ytmp[64 * h:64 * h + 64, :],
                                              yh[h][0:64, :])
                        nc.vector.tensor_copy(lrow[:], yh[h][64:65, :])
                        rec = rcp.tile([1, 512], F32, tag="rec",
                                       name="rec", bufs=8)
                        nc.vector.reciprocal_approx_fast(rec[:], lrow[:])
                        rb = rcp.tile([128, 512], F32, tag="rb", bufs=8,
                                      name="rb")
                        nc.gpsimd.partition_broadcast(rb[:], rec[:])

                        def norm(h=h, ytmp=ytmp, yst=yst, rb=rb, ns=ns,
                                 hp=hp):
                            if ns == 0:
                                nc.vector.tensor_mul(
                                    yst[64 * h:64 * h + 64, 0:256],
                                    ytmp[64 * h:64 * h + 64, 0:256],
                                    rb[64 * h:64 * h + 64, 0:256])
                                nc.vector.tensor_mul(
                                    yT[64 * h:64 * h + 64, hp, 256:512],
                                    ytmp[64 * h:64 * h + 64, 256:512],
                                    rb[64 * h:64 * h + 64, 256:512])
                            else:
                                nc.vector.tensor_mul(
                                    yst[64 * h:64 * h + 64, :],
                                    ytmp[64 * h:64 * h + 64, :],
                                    rb[64 * h:64 * h + 64, :])
                        nkey = (tick(o + 2, 0) + h if o + 2 < NORD
                                else tick(NORD - 1, 2) + h)
                        add(nkey, norm)
                return flush

            if o < NORD - 1:
                flushes[o] = make_flush()
            else:
                # last ordinal: flush inline, interleaving the final
                # normalizes with the last strip's proj units
                pop_due(F(o))
                while prevs:
                    emit_yT(*prevs.popleft())
                yst = yT[:, hp, q0:q0 + 512]
                ytmp = ytp.tile([128, 512], BF16, tag="ytmp", name="ytmp")
                rbs = []
                for h in range(2):
                    lrow = rcp.tile([1, 512], F32, tag="lrow", name="lrow",
                                    bufs=8)
                    nc.scalar.copy(ytmp[64 * h:64 * h + 64, :],
                                   yh[h][0:64, :])
                    nc.scalar.copy(lrow[:], yh[h][64:65, :])
                    rec = rcp.tile([1, 512], F32, tag="rec", name="rec",
                                   bufs=8)
                    nc.vector.reciprocal_approx_fast(rec[:], lrow[:])
                    rb = rcp.tile([128, 512], F32, tag="rb", bufs=8,
                                  name="rb")
                    rbs.append(rb)
                    nc.gpsimd.partition_broadcast(rb[:], rec[:])
                for qc in range(4):
                    cl, cr = 128 * qc, 128 * qc + 128
                    for h in range(2):
                        nc.vector.tensor_mul(
                            yst[64 * h:64 * h + 64, cl:cr],
                            ytmp[64 * h:64 * h + 64, cl:cr],
                            rbs[h][64 * h:64 * h + 64, cl:cr])
                    p4b_unit(4 * ns + qc)
        while work:
            work.pop(0)[3]()

    nc.finalize()
    return nc


def shard_inputs(x, w_qkv, b_qkv, w_proj):
    """Host-side sharding: returns list of per-core input dicts.

      x8  [128, ns, d, s]  fp8   w8  [128, d, 1152] fp8 (x16, qk-pair-major)
      x0b [128, d, s0]     bf16  wvb [128, d, 384]  bf16 (x16)
      wp8/wpb [128, 3, 768] (x16), bqk [128, 6] f32 (x16)
    """
    import ml_dtypes
    E4M3 = ml_dtypes.float8_e4m3fn
    BF = ml_dtypes.bfloat16
    S16 = np.float32(16.0)
    in_maps = []
    for core in range(NCORES):
        b, hg = (core // 2) % x.shape[0], core % 2
        cs = slice(384 * hg, 384 * hg + 384)
        xT_s = np.ascontiguousarray(x[b].T).astype(np.float32)  # [768, 2048]
        q_s = w_qkv[:, 0:768][:, cs]
        k_s = w_qkv[:, 768:1536][:, cs]
        v_s = w_qkv[:, 1536:2304][:, cs]
        # qk-pair-major: [q0|k0|q1|k1|q2|k2|v]
        wqkv_s = np.concatenate(
            [q_s[:, 0:128], k_s[:, 0:128], q_s[:, 128:256], k_s[:, 128:256],
             q_s[:, 256:384], k_s[:, 256:384], v_s], axis=1) * S16
        bqk_c = np.concatenate([b_qkv[0:768][cs], b_qkv[768:1536][cs]])
        bqk_s = np.ascontiguousarray(bqk_c.reshape(6, 128).T) * S16
        wproj_s = w_proj[384 * hg:384 * hg + 384, :] * S16  # [384, 768]

        x8 = xT_s.reshape(6, 128, 4, 512).transpose(1, 2, 0, 3)
        w8 = wqkv_s.reshape(6, 128, 1152).transpose(1, 0, 2)
        x0b = xT_s[:, 0:256].reshape(6, 128, 256).transpose(1, 0, 2)
        wvb_h = wqkv_s[:, 768:1152].reshape(6, 128, 384).transpose(1, 0, 2)
        wpp = wproj_s.reshape(3, 128, 768).transpose(1, 0, 2)
        in_maps.append({
            "x8": np.ascontiguousarray(x8).astype(E4M3).reshape(128, -1),
            "w8": np.ascontiguousarray(w8).astype(E4M3),
            "x0b": np.ascontiguousarray(x0b).astype(BF).reshape(128, -1),
            "wvb": np.ascontiguousarray(wvb_h).astype(BF).reshape(128, -1),
            "bqk_s": bqk_s.astype(np.float32),
            "wp8": np.ascontiguousarray(wpp).astype(E4M3).reshape(128, -1),
            "wpb": np.ascontiguousarray(wpp).astype(BF).reshape(128, -1),
        })
    return in_maps


_CACHED = {}


def _get_program():
    if "nc" not in _CACHED:
        _CACHED["nc"] = build_program()
    return _CACHED["nc"]


def _spot_check(outp, x, w_qkv, b_qkv, w_proj, b_proj):
    """Exact per-row reference on a few rows; returns worst relative error.
    Guards against rare transient bad compiles/executions."""
    B, S, dim = x.shape
    H, HD = 12, 64
    worst = 0.0
    checks = [(b, min(S - 1, 511 + 512 * b)) for b in range(B)]
    checks += [(0, 5), (1, 300), (2, 1200), (3, 1800)]
    for b, s in checks:
        xb = x[b].astype(np.float64)
        q = xb[s] @ w_qkv[:, 0:768] + b_qkv[0:768]
        k = xb[:s + 1] @ w_qkv[:, 768:1536] + b_qkv[768:1536]
        v = xb[:s + 1] @ w_qkv[:, 1536:2304] + b_qkv[1536:2304]
        ys = []
        for h in range(H):
            sc = (k[:, HD * h:HD * h + HD] @ q[HD * h:HD * h + HD]) * 0.125
            e = np.exp(sc - sc.max())
            ys.append((e / e.sum()) @ v[:, HD * h:HD * h + HD])
        row = np.concatenate(ys) @ w_proj + b_proj
        rel = np.abs(outp[b, s] - row).max() / max(np.abs(row).max(), 1e-6)
        worst = max(worst, rel)
    return worst


def kernel(x, w_qkv, b_qkv, w_proj, b_proj):
    import jax
    from concourse.bass_utils import run_bass_kernel_spmd

    x = np.asarray(x, dtype=np.float32)
    w_qkv = np.asarray(w_qkv, dtype=np.float32)
    b_qkv = np.asarray(b_qkv, dtype=np.float32)
    w_proj = np.asarray(w_proj, dtype=np.float32)
    b_proj = np.asarray(b_proj, dtype=np.float32)

    B, S, dim = x.shape
    in_maps = shard_inputs(x, w_qkv, b_qkv, w_proj)
    # v-bias folds out of attention (rows of attn sum to exactly 1)
    bvw = b_qkv[1536:2304] @ w_proj  # [D]
    const_row = (b_proj + bvw)[None, :]

    outp = np.empty((B, S, dim), dtype=np.float32)
    for attempt in range(3):
        nc = _get_program()
        res = run_bass_kernel_spmd(nc, in_maps, core_ids=list(range(NCORES)))
        parts = [m["out_s"] for m in res.results]
        for b in range(B):
            outp[b] = parts[2 * b] + parts[2 * b + 1] + const_row
        if _spot_check(outp, x, w_qkv, b_qkv, w_proj, b_proj) < 1.2e-2:
            break
        # transient bad build/execution: clear caches, rebuild, rerun
        _CACHED.clear()
        jax.clear_caches()
    return outp
